# revision 17
# baseline (speedup 1.0000x reference)
"""Trainium2 Bass kernel: fused GRU + interaction double-recurrence (ANIMA).

Strategy: data-parallel over batch (8 rows/core on 8 cores). Everything runs in
transposed [feature, batch] layout (feature on partitions). One main For_i loop
over blocks of 32 timesteps fuses all phases; per-step emission is ordered so
the interaction chain (the critical path: De -> tanh -> phi_D -> sigmoid -> Dc
-> tanh -> De) is never queued behind non-critical work on PE/ACT/DVE, and the
GRU + encode/head matmuls fill its stall windows.
  body j: [obs dma j+1] [s-loop: GRU(j) + interaction(j-1) interleaved]
          [C: P_SM(j)] [E: Sn/Mn + heads (j-1)] [A: encode/precompute (j+1)]
Intermediates never round-trip through DRAM; DMA is only obs in / outputs out.
"""
import sys
sys.path.insert(0, '/opt/trn_rl_repo')
import numpy as np
import ml_dtypes

import concourse.bass as bass
import concourse.bacc as bacc
import concourse.mybir as mybir
from concourse import tile
from concourse.bass_utils import run_bass_kernel_spmd

BF16 = ml_dtypes.bfloat16
F32 = mybir.dt.float32
BF = mybir.dt.bfloat16
AF = mybir.ActivationFunctionType
ds = bass.ds

B, T, D_S, D_M, D_B, D_ST, D_O, CHUNK = 64, 2048, 64, 256, 128, 256, 64, 16
NCORES = 8
BS = B // NCORES            # batch rows per core
SPB = 64                    # timesteps per block
COLS = SPB * BS             # sbuf columns per block

# bias column registry in the packed [128, NBIAS] f32 bias tensor
_BC = {}
_nb = 0
for _name, _n in [('we_b', 2), ('wc_b', 1), ('wz_b', 2), ('wr_b', 2), ('wh_b', 2),
                  ('phi_b', 5), ('dck_b', 1), ('sck_b', 1), ('mck_b', 1),
                  ('se_b', 2), ('me_b', 2), ('dc2_b', 1), ('de2_b', 2),
                  ('p1_b', 1), ('oh_b', 1), ('p2_b', 1)]:
    _BC[_name] = _nb
    _nb += _n
NBIAS = _nb

# weight param shapes (bf16 lhsT tile grids)
WSHAPES = {
    'we':    [64, 2, 128],
    'wc':    [128, 2, 128],
    'wzx':   [128, 2, 2, 128], 'wrx': [128, 2, 2, 128], 'whx': [128, 2, 2, 128],
    'wzh':   [128, 2, 2, 128], 'wrh': [128, 2, 2, 128], 'whh': [128, 2, 2, 128],
    'wp':    [128, 2, 2, 128],
    'phism': [128, 3, 5, 128],
    'phid':  [128, 2, 5, 128],
    'dck':   [128, 5, 128], 'sck': [128, 5, 128], 'mck': [128, 5, 128],
    'sew':   [128, 2, 128], 'mew': [128, 2, 128], 'dew': [128, 2, 128],
    'dc2':   [128, 6, 128],
    'de2':   [128, 2, 128],
    'ohw':   [128, 2, 64],
    'p1w':   [128, 4, 128],
    'p2w':   [128, 64],
}


def build(T_=T):
    NBLK = T_ // SPB
    nc = bacc.Bacc("TRN2", target_bir_lowering=False, debug=False, num_devices=NCORES)

    obsT = nc.declare_dram_parameter("obsT", [D_S, (T_ + 2 * SPB) * BS], BF, isOutput=False)
    wdr = {k: nc.declare_dram_parameter(k, sh, BF, isOutput=False) for k, sh in WSHAPES.items()}
    bdr = nc.declare_dram_parameter("biases", [128, NBIAS], F32, isOutput=False)
    idr = nc.declare_dram_parameter("ident", [128, 128], BF, isOutput=False)
    # brows: K-row biases for PE-side bias injection: [which(0=De_b,1=Wp_b), 2, 128]
    brdr = nc.declare_dram_parameter("brows", [2, 2, 128], BF, isOutput=False)
    seldr = nc.declare_dram_parameter("sel", [2, 16], BF, isOutput=False)
    actT = nc.declare_dram_parameter("actT", [D_O, T_ * BS], F32, isOutput=True)
    predT = nc.declare_dram_parameter("predT", [D_S, T_ * BS], F32, isOutput=True)
    finals = nc.declare_dram_parameter("finals", [128, 48], F32, isOutput=True)

    with tile.TileContext(nc) as tc:
        with (
            tc.tile_pool(name="w", bufs=1) as wp,
            tc.tile_pool(name="st", bufs=1) as sp,
            tc.tile_pool(name="kp", bufs=3) as kp,
            tc.tile_pool(name="pss", bufs=2, space="PSUM") as pss,
            tc.tile_pool(name="psb", bufs=2, space="PSUM") as psb,
        ):
            # ---- resident weights & biases
            W = {}
            for k, sh in WSHAPES.items():
                W[k] = wp.tile(sh, BF, tag=k, name=f'w_{k}')
                nc.sync.dma_start(W[k][...], wdr[k][...])
            bia = wp.tile([128, NBIAS], F32, tag='bias')
            nc.sync.dma_start(bia[...], bdr[...])
            ident = wp.tile([128, 128], BF, tag='ident')
            nc.sync.dma_start(ident[...], idr[...])
            brows = wp.tile([2, 2, 128], BF, tag='brows')
            nc.sync.dma_start(brows[...], brdr[...])
            # selector rhs for K=2 bias matmuls: sel[k, c] = 1 iff c//8 == k
            sel = wp.tile([2, 16], BF, tag='sel')
            nc.sync.dma_start(sel[...], seldr[...])

            def bc(name, i=0):
                return bia[:, _BC[name] + i: _BC[name] + i + 1]

            def bc64(name):
                return bia[0:64, _BC[name]: _BC[name] + 1]

            # ---- persistent block buffers
            obs_sb = sp.tile([D_S, COLS], BF, tag='obs')
            sen = sp.tile([128, 2, SPB, BS], BF, tag='sen')
            xzr = sp.tile([128, SPB, 32], BF, tag='xzr')
            xh = sp.tile([128, SPB, 16], BF, tag='xh')
            slow_c = sp.tile([128, SPB, BS], BF, tag='slowc')
            m_c = sp.tile([128, SPB, 16], BF, tag='mc')
            psm = sp.tile([128, SPB, 40], BF, tag='psm')
            gch = sp.tile([128, SPB, 40], BF, tag='gch')
            # comb: prev-block [S_low | M] in cols 0:24; D-state ring in cols 24:40
            # (slot s holds D(s-1); slot SPB holds D(SPB-1) for the wrap copy)
            comb = sp.tile([128, SPB + 1, 40], BF, tag='comb')
            h_car = sp.tile([128, 16], BF, tag='hcar')
            snch = sp.tile([128, 2, SPB, BS], BF, tag='snch')
            mnch = sp.tile([128, 2, SPB, BS], BF, tag='mnch')

            def r2(ap, m=2):
                return ap.rearrange("p (m c) -> p m c", m=m)

            # ========== per-step pieces ==========
            def h_prev_ap(s):
                return h_car[:, :] if s % CHUNK == 0 else m_c[:, s - 1]

            def d_prev_ap(s):
                return comb[:, s, 24:40]

            # --- PE emissions (order matters: these define the PE queue) ---
            def pe_phid(s, p_ps):
                dp = d_prev_ap(s)
                for m in range(5):
                    for k in range(2):
                        nc.tensor.matmul(p_ps[:, m], W['phid'][:, k, m, :],
                                         dp[:, 8 * k: 8 * k + 8], start=False,
                                         stop=(m == 4 and k == 1))

            def pe_zr(s, zr_ps):
                hp = h_prev_ap(s)
                for gi, wname in ((0, 'wzh'), (1, 'wrh')):
                    for m in range(2):
                        for k in range(2):
                            nc.tensor.matmul(zr_ps[:, 2 * gi + m], W[wname][:, k, m, :],
                                             hp[:, 8 * k: 8 * k + 8],
                                             start=False,
                                             stop=(gi == 1 and m == 1 and k == 1))

            def pe_dc(s):
                a_ps = pss.tile([128, BS], F32, tag='stepI', name='a_ps')
                for k in range(5):
                    nc.tensor.matmul(a_ps[...], W['dck'][:, k, :], gch[:, s, 8 * k: 8 * k + 8],
                                     start=(k == 0), stop=(k == 4))
                return a_ps

            def pe_hc(s, rh, hc_ps):
                for m in range(2):
                    for k in range(2):
                        nc.tensor.matmul(hc_ps[:, m], W['whh'][:, k, m, :],
                                         rh[:, 8 * k: 8 * k + 8], start=False,
                                         stop=(m == 1 and k == 1))

            def pe_de(s, a_bf, d_ps):
                for m in range(2):
                    nc.tensor.matmul(d_ps[:, m], W['dew'][:, m, :], a_bf[...],
                                     start=False, stop=True)

            def pe_prop(s, hp_ps):
                for m in range(2):
                    for k in range(2):
                        nc.tensor.matmul(hp_ps[:, m], W['wp'][:, k, m, :],
                                         m_c[:, s, 8 * k: 8 * k + 8],
                                         start=False, stop=(m == 1 and k == 1))

            # ========== fused s-loop (GRU step s of block j; interaction step s of j-1) ==========
            def fused_steps(with_int=True, with_gru=True, s_lo=0, s_hi=SPB):
                for s in range(s_lo, s_hi):
                    prop = with_gru and (s % CHUNK == CHUNK - 1)
                    # --- early PSUM injections (no chain deps): streams + biases
                    if with_int:
                        p_ps = pss.tile([128, 5, BS], F32, tag='stepP', name='p_ps')
                        nc.tensor.matmul(p_ps[...], ident[...], psm[:, s],
                                         start=True, stop=False)
                        d_ps = pss.tile([128, 2, BS], F32, tag='stepI', name='d_ps')
                        nc.tensor.matmul(d_ps[...], brows[:, 0], sel[...],
                                         start=True, stop=False)
                    if with_gru:
                        zr_ps = pss.tile([128, 4, BS], F32, tag='stepG', name='zr_ps')
                        nc.tensor.matmul(zr_ps[...], ident[...], xzr[:, s],
                                         start=True, stop=False)
                        hc_ps = pss.tile([128, 2, BS], F32, tag='stepG', name='hc_ps')
                        nc.tensor.matmul(hc_ps[...], ident[...], xh[:, s],
                                         start=True, stop=False)
                        if prop:
                            hp_ps = pss.tile([128, 2, BS], F32, tag='stepG', name='hp_ps')
                            nc.tensor.matmul(hp_ps[...], brows[:, 1], sel[...],
                                             start=True, stop=False)
                    # --- recurrent matmul groups
                    if with_int:
                        pe_phid(s, p_ps)
                    if with_gru:
                        pe_zr(s, zr_ps)
                    # --- sigmoids straight from PSUM
                    if with_int:
                        sigp = kp.tile([128, 5, BS], F32, tag='sigp', name='sigp')
                        nc.scalar.activation(sigp[...], p_ps[...], AF.Sigmoid)
                        nc.vector.tensor_mul(r2(gch[:, s], 5), sigp[...], r2(comb[:, s], 5))
                    if with_gru:
                        sig = kp.tile([128, 4, BS], F32, tag='sigzr', name='sig')
                        nc.scalar.activation(sig[...], zr_ps[...], AF.Sigmoid)
                        hp = h_prev_ap(s)
                        rh = kp.tile([128, 16], BF, tag='rh', name='rh')
                        nc.vector.tensor_mul(r2(rh), sig[:, 2:4], r2(hp))
                    # --- second matmul wave
                    a_ps = pe_dc(s) if with_int else None
                    if with_gru:
                        pe_hc(s, rh, hc_ps)
                    if with_int:
                        a_bf = kp.tile([128, BS], BF, tag='abf', name='a_bf')
                        nc.scalar.activation(a_bf[...], a_ps[...], AF.Tanh, bias=bc('dck_b'))
                    if with_gru:
                        # t4 = (z - 1) * h  (overlaps hc matmuls)
                        t4 = kp.tile([128, 2, BS], F32, tag='gtmp', name='t4')
                        nc.vector.scalar_tensor_tensor(
                            t4[...], sig[:, 0:2], 1.0, r2(hp),
                            mybir.AluOpType.subtract, mybir.AluOpType.mult)
                    # --- final matmul + activations
                    if with_int:
                        pe_de(s, a_bf, d_ps)
                        nc.scalar.activation(r2(comb[:, s + 1, 24:40]), d_ps[...], AF.Tanh)
                    if with_gru:
                        hc = kp.tile([128, 2, BS], F32, tag='hc', name='hc')
                        nc.scalar.activation(hc[...], hc_ps[...], AF.Tanh)
                        t3 = kp.tile([128, 2, BS], F32, tag='gtmp', name='t3')
                        nc.vector.tensor_mul(t3[...], sig[:, 0:2], hc[...])
                        nc.vector.tensor_sub(r2(m_c[:, s]), t3[...], t4[...])
                        if prop:
                            pe_prop(s, hp_ps)
                            nc.scalar.activation(r2(h_car[:, :]), hp_ps[...], AF.Tanh)

            # ========== stage C piece: one P_SM matmul group ==========
            def c_group(m):
                ps = psb.tile([128, SPB, BS], F32, tag='blk', name='c_ps')
                nc.tensor.matmul(ps[...], W['phism'][:, 0, m, :], slow_c[...], start=True, stop=False)
                for kk in (1, 2):
                    nc.tensor.matmul(ps[...], W['phism'][:, kk, m, :],
                                     m_c[:, :, 8 * (kk - 1): 8 * kk],
                                     start=False, stop=(kk == 2))
                nc.vector.tensor_scalar_add(psm[:, :, 8 * m: 8 * m + 8], ps[...], bc('phi_b', m))

            # ========== stage A pieces: encode + GRU input precompute ==========
            def a_sen():
                for m in range(2):
                    ps = psb.tile([128, SPB, BS], F32, tag='blk', name='a_ps')
                    nc.tensor.matmul(ps[...], W['we'][:, m, :], obs_sb[...], start=True, stop=True)
                    nc.scalar.activation(sen[:, m], ps[...], AF.Tanh, bias=bc('we_b', m))

            def a_x(wname, dst, base, bname):
                for m in range(2):
                    ps = psb.tile([128, SPB, BS], F32, tag='blk', name='a_ps2')
                    for k in range(2):
                        nc.tensor.matmul(ps[...], W[wname][:, k, m, :], sen[:, k],
                                         start=(k == 0), stop=(k == 1))
                    nc.vector.tensor_scalar_add(
                        dst[:, :, base + 8 * m: base + 8 * (m + 1)], ps[...], bc(bname, m))

            def a_wc():
                ps = psb.tile([128, SPB, BS], F32, tag='blk', name='a_ps3')
                for k in range(2):
                    nc.tensor.matmul(ps[...], W['wc'][:, k, :], sen[:, k],
                                     start=(k == 0), stop=(k == 1))
                nc.scalar.activation(slow_c[...], ps[...], AF.Tanh, bias=bc('wc_b'))

            # ========== stage E pieces: deferred Sn/Mn + heads ==========
            def e_cmm(cw, cb):
                ps = psb.tile([128, SPB, BS], F32, tag='blk', name='e_ps')
                for k in range(5):
                    nc.tensor.matmul(ps[...], W[cw][:, k, :], gch[:, :, 8 * k: 8 * k + 8],
                                     start=(k == 0), stop=(k == 4))
                t1 = kp.tile([128, SPB, BS], BF, tag='t1', name='t1')
                nc.scalar.activation(t1[...], ps[...], AF.Tanh, bias=bc(cb))
                return t1

            def e_emm(ew, eb, t1, dst):
                for m in range(2):
                    ps2 = psb.tile([128, SPB, BS], F32, tag='blk', name='e_ps2')
                    nc.tensor.matmul(ps2[...], W[ew][:, m, :], t1[...], start=True, stop=True)
                    nc.scalar.activation(dst[:, m], ps2[...], AF.Tanh, bias=bc(eb, m))

            def rhs6():
                return [snch[:, 0], snch[:, 1], mnch[:, 0], mnch[:, 1],
                        comb[:, 1:SPB + 1, 24:32], comb[:, 1:SPB + 1, 32:40]]

            def e_dc2():
                ps = psb.tile([128, SPB, BS], F32, tag='blk', name='e_ps3')
                for k, r in enumerate(rhs6()):
                    nc.tensor.matmul(ps[...], W['dc2'][:, k, :], r,
                                     start=(k == 0), stop=(k == 5))
                t1 = kp.tile([128, SPB, BS], BF, tag='t1', name='t1b')
                nc.scalar.activation(t1[...], ps[...], AF.Tanh, bias=bc('dc2_b'))
                return t1

            def e_de2(t1):
                dec = kp.tile([128, 2, SPB, BS], BF, tag='dec', name='dec')
                for m in range(2):
                    ps2 = psb.tile([128, SPB, BS], F32, tag='blk', name='e_ps4')
                    nc.tensor.matmul(ps2[...], W['de2'][:, m, :], t1[...], start=True, stop=True)
                    nc.scalar.activation(dec[:, m], ps2[...], AF.Tanh, bias=bc('de2_b', m))
                return dec

            def e_oh(dec, out_off):
                ps3 = psb.tile([64, SPB, BS], F32, tag='blk', name='e_ps5')
                for k in range(2):
                    nc.tensor.matmul(ps3[...], W['ohw'][:, k, :], dec[:, k],
                                     start=(k == 0), stop=(k == 1))
                act_o = kp.tile([64, SPB, BS], F32, tag='acto', name='act_o')
                nc.vector.tensor_scalar_add(act_o[...], ps3[...], bc64('oh_b'))
                nc.sync.dma_start(actT[:, ds(out_off, COLS)], act_o[...])

            def e_p1():
                ps = psb.tile([128, SPB, BS], F32, tag='blk', name='e_ps6')
                for k, r in enumerate(rhs6()[:4]):
                    nc.tensor.matmul(ps[...], W['p1w'][:, k, :], r,
                                     start=(k == 0), stop=(k == 3))
                t1 = kp.tile([128, SPB, BS], BF, tag='t1', name='t1c')
                nc.scalar.activation(t1[...], ps[...], AF.Tanh, bias=bc('p1_b'))
                return t1

            def e_p2(t1, out_off):
                ps3 = psb.tile([64, SPB, BS], F32, tag='blk', name='e_ps7')
                nc.tensor.matmul(ps3[...], W['p2w'][...], t1[...], start=True, stop=True)
                pred_o = kp.tile([64, SPB, BS], F32, tag='predo', name='pred_o')
                nc.vector.tensor_scalar_add(pred_o[...], ps3[...], bc64('p2_b'))
                nc.sync.dma_start(predT[:, ds(out_off, COLS)], pred_o[...])

            def stage_CEA(out_off, with_e=True, with_a=True, with_c=True):
                """Interleaved tail: C groups fill E's activation waits, A fills the rest."""
                if with_c:
                    for m in range(3):
                        c_group(m)
                if with_e:
                    t1s = e_cmm('sck', 'sck_b')
                if with_c:
                    c_group(3)
                if with_e:
                    t1m = e_cmm('mck', 'mck_b')
                if with_c:
                    c_group(4)
                    copies()
                if with_a:
                    a_sen()
                if with_e:
                    e_emm('sew', 'se_b', t1s, snch)
                    e_emm('mew', 'me_b', t1m, mnch)
                if with_a:
                    a_x('wzx', xzr, 0, 'wz_b')
                if with_e:
                    t1d = e_dc2()
                if with_a:
                    a_x('wrx', xzr, 16, 'wr_b')
                if with_e:
                    dec = e_de2(t1d)
                if with_a:
                    a_x('whx', xh, 0, 'wh_b')
                if with_e:
                    e_oh(dec, out_off)
                    t1p = e_p1()
                if with_a:
                    a_wc()
                if with_e:
                    e_p2(t1p, out_off)

            def copies():
                nc.vector.tensor_copy(comb[:, 0:SPB, 0:8], slow_c[...])
                nc.vector.tensor_copy(comb[:, 0:SPB, 8:24], m_c[...])
                nc.vector.tensor_copy(comb[:, 0:1, 24:40], comb[:, SPB:SPB + 1, 24:40])

            # ================= prologue: block 0 (+ A(1)) =================
            nc.vector.memset(h_car[...], 0.0)
            nc.vector.memset(comb[...], 0.0)
            nc.sync.dma_start(obs_sb[...], obsT[:, 0:COLS])
            a_sen()
            a_x('wzx', xzr, 0, 'wz_b')
            a_x('wrx', xzr, 16, 'wr_b')
            a_x('whx', xh, 0, 'wh_b')
            a_wc()                         # A(0)
            nc.sync.dma_start(obs_sb[...], obsT[:, COLS: 2 * COLS])
            fused_steps(with_int=False)    # B(0)
            stage_CEA(0, with_e=False)     # C(0) + copies + A(1)

            # ================= main loop: j in [1, NBLK) =================
            with tc.For_i(1, NBLK, 1, staggered_reset=True,
                          hint_engines=(mybir.EngineType.PE, mybir.EngineType.DVE,
                                        mybir.EngineType.Activation)) as j:
                nc.sync.dma_start(obs_sb[...], obsT[:, ds((j + 1) * COLS, COLS)])
                fused_steps(s_hi=SPB // 2)           # first half of B(j) + D(j-1)
                tc.stage_boundary()
                fused_steps(s_lo=SPB // 2)           # second half
                tc.stage_boundary()
                for _m in range(5):
                    c_group(_m)
                copies()
                tc.stage_boundary()
                stage_CEA((j - 1) * COLS, with_c=False)  # E(j-1) + A(j+1)

            # ================= epilogue: interaction + heads for last block =================
            fused_steps(with_gru=False)    # D(NBLK-1)
            t1s = e_cmm('sck', 'sck_b')
            t1m = e_cmm('mck', 'mck_b')
            e_emm('sew', 'se_b', t1s, snch)
            e_emm('mew', 'me_b', t1m, mnch)
            t1d = e_dc2()
            dec = e_de2(t1d)
            e_oh(dec, (NBLK - 1) * COLS)
            t1p = e_p1()
            e_p2(t1p, (NBLK - 1) * COLS)
            nc.gpsimd.dma_start(finals[:, 0:8], snch[:, 0, SPB - 1])
            nc.gpsimd.dma_start(finals[:, 8:16], snch[:, 1, SPB - 1])
            nc.gpsimd.dma_start(finals[:, 16:24], mnch[:, 0, SPB - 1])
            nc.gpsimd.dma_start(finals[:, 24:32], mnch[:, 1, SPB - 1])
            nc.gpsimd.dma_start(finals[:, 32:40], comb[:, SPB, 24:32])
            nc.gpsimd.dma_start(finals[:, 40:48], comb[:, SPB, 32:40])

    nc.compile()
    return nc


# ======================= host-side packing =======================

def _tiles(Wm, kt, mt, mw=128):
    """[K, M] f32 -> [128, kt, mt, mw] bf16 lhsT tile grid (squeezed per caller)."""
    out = np.zeros((128, kt, mt, mw), np.float32)
    for k in range(kt):
        for m in range(mt):
            blk = Wm[k * 128:(k + 1) * 128, m * mw:(m + 1) * mw]
            out[:blk.shape[0], k, m, :blk.shape[1]] = blk
    return out.astype(BF16)


def _pack_weights(p):
    KEEP = np.r_[0:128, 256:768]
    phi = p['phi']
    w = {}
    w['we'] = _tiles(p['We'], 1, 2)[0:64, 0]                      # [64,2,128]
    w['wc'] = _tiles(p['Wc'], 2, 1)[:, :, 0]                      # [128,2,128]
    for nm, mat in (('wzx', p['Wz'][:D_M]), ('wrx', p['Wr'][:D_M]), ('whx', p['Wh'][:D_M]),
                    ('wzh', p['Wz'][D_M:]), ('wrh', p['Wr'][D_M:]), ('whh', p['Wh'][D_M:]),
                    ('wp', p['Wp'])):
        w[nm] = _tiles(mat, 2, 2)
    phism = np.concatenate([phi[0:128], phi[256:512]], axis=0)[:, KEEP]   # [384, 640]
    w['phism'] = _tiles(phism, 3, 5)
    w['phid'] = _tiles(phi[512:768][:, KEEP], 2, 5)
    w['dck'] = _tiles(p['Dc'][KEEP], 5, 1)[:, :, 0]
    w['sck'] = _tiles(p['Sc'][KEEP], 5, 1)[:, :, 0]
    w['mck'] = _tiles(p['Mc'][KEEP], 5, 1)[:, :, 0]
    w['sew'] = _tiles(p['Se'], 1, 2)[:, 0]
    w['mew'] = _tiles(p['Me'], 1, 2)[:, 0]
    w['dew'] = _tiles(p['De'], 1, 2)[:, 0]
    w['dc2'] = _tiles(p['dc'], 6, 1)[:, :, 0]
    w['de2'] = _tiles(p['de'], 1, 2)[:, 0]
    w['ohw'] = _tiles(p['oh'], 2, 1, mw=64)[:, :, 0]
    w['p1w'] = _tiles(p['p1'], 4, 1)[:, :, 0]
    w['p2w'] = _tiles(p['p2'], 1, 1, mw=64)[:, 0, 0]
    for k, sh in WSHAPES.items():
        assert list(w[k].shape) == sh, (k, w[k].shape, sh)
    return w


def _pack_biases(p):
    KEEP = np.r_[0:128, 256:768]
    bia = np.zeros((128, NBIAS), np.float32)

    def put(name, vec):
        vec = np.asarray(vec, np.float32)
        n = (len(vec) + 127) // 128
        for i in range(n):
            seg = vec[i * 128:(i + 1) * 128]
            bia[:len(seg), _BC[name] + i] = seg
    put('we_b', p['We_b']); put('wc_b', p['Wc_b'])
    put('wz_b', p['Wz_b']); put('wr_b', p['Wr_b']); put('wh_b', p['Wh_b'])
    put('phi_b', np.asarray(p['phi_b'])[KEEP])
    put('dck_b', p['Dc_b']); put('sck_b', p['Sc_b']); put('mck_b', p['Mc_b'])
    put('se_b', p['Se_b']); put('me_b', p['Me_b'])
    put('dc2_b', p['dc_b']); put('de2_b', p['de_b'])
    put('p1_b', p['p1_b']); put('oh_b', p['oh_b']); put('p2_b', p['p2_b'])
    return bia


def _pack_brows(p):
    # [k_row(=m tile), which(0=De_b,1=Wp_b), 128]: lhsT slice brows[:, w] is [2, 128]
    br = np.zeros((2, 2, 128), np.float32)
    for m in range(2):
        br[m, 0] = np.asarray(p['De_b'], np.float32)[m * 128:(m + 1) * 128]
        br[m, 1] = np.asarray(p['Wp_b'], np.float32)[m * 128:(m + 1) * 128]
    return br.astype(BF16)


_NC_CACHE = {}


def _get_nc(T_):
    if T_ not in _NC_CACHE:
        _NC_CACHE[T_] = build(T_)
    return _NC_CACHE[T_]


def kernel(obs_seq, params):
    obs = np.asarray(obs_seq, np.float32)
    p = {k: np.asarray(v, np.float32) for k, v in params.items()}
    Bf, Tf = obs.shape[0], obs.shape[1]
    nc = _get_nc(Tf)

    common = _pack_weights(p)
    common['biases'] = _pack_biases(p)
    common['ident'] = np.eye(128, dtype=np.float32).astype(BF16)
    selm = np.zeros((2, 16), np.float32)
    selm[0, 0:8] = 1.0
    selm[1, 8:16] = 1.0
    common['sel'] = selm.astype(BF16)
    common['brows'] = _pack_brows(p)

    in_maps = []
    for c in range(NCORES):
        shard = obs[c * BS:(c + 1) * BS]                      # [BS, T, D_S]
        obsT = shard.transpose(2, 1, 0).reshape(D_S, Tf * BS)  # col = t*BS + b
        pad = np.zeros((D_S, 2 * SPB * BS), np.float32)
        obsT = np.concatenate([obsT, pad], axis=1).astype(BF16)
        im = dict(common)
        im['obsT'] = obsT
        in_maps.append(im)

    res = run_bass_kernel_spmd(nc, in_maps, core_ids=list(range(NCORES)))

    actions = np.zeros((Bf, Tf, D_O), np.float32)
    preds = np.zeros((Bf, Tf, D_S), np.float32)
    S_last = np.zeros((Bf, D_ST), np.float32)
    M_last = np.zeros((Bf, D_ST), np.float32)
    D_last = np.zeros((Bf, D_ST), np.float32)
    for c in range(NCORES):
        r = res.results[c]
        sl = slice(c * BS, (c + 1) * BS)
        actions[sl] = r['actT'].reshape(D_O, Tf, BS).transpose(2, 1, 0)
        preds[sl] = r['predT'].reshape(D_S, Tf, BS).transpose(2, 1, 0)
        fin = r['finals']                                     # [128, 48]
        S_last[sl] = fin[:, 0:16].reshape(128, 2, BS).transpose(2, 1, 0).reshape(BS, 256)
        M_last[sl] = fin[:, 16:32].reshape(128, 2, BS).transpose(2, 1, 0).reshape(BS, 256)
        D_last[sl] = fin[:, 32:48].reshape(128, 2, BS).transpose(2, 1, 0).reshape(BS, 256)
    return actions, preds, S_last, M_last, D_last


# revision 18
# speedup vs baseline: 1.0186x; 1.0186x over previous
"""Trainium2 Bass kernel: fused GRU + interaction double-recurrence (ANIMA).

Strategy: data-parallel over batch (8 rows/core on 8 cores). Everything runs in
transposed [feature, batch] layout (feature on partitions). One main For_i loop
over blocks of 32 timesteps fuses all phases; per-step emission is ordered so
the interaction chain (the critical path: De -> tanh -> phi_D -> sigmoid -> Dc
-> tanh -> De) is never queued behind non-critical work on PE/ACT/DVE, and the
GRU + encode/head matmuls fill its stall windows.
  body j: [obs dma j+1] [s-loop: GRU(j) + interaction(j-1) interleaved]
          [C: P_SM(j)] [E: Sn/Mn + heads (j-1)] [A: encode/precompute (j+1)]
Intermediates never round-trip through DRAM; DMA is only obs in / outputs out.
"""
import sys
sys.path.insert(0, '/opt/trn_rl_repo')
import numpy as np
import ml_dtypes

import concourse.bass as bass
import concourse.bacc as bacc
import concourse.mybir as mybir
from concourse import tile
from concourse.bass_utils import run_bass_kernel_spmd

BF16 = ml_dtypes.bfloat16
F32 = mybir.dt.float32
BF = mybir.dt.bfloat16
AF = mybir.ActivationFunctionType
ds = bass.ds

B, T, D_S, D_M, D_B, D_ST, D_O, CHUNK = 64, 2048, 64, 256, 128, 256, 64, 16
NCORES = 8
BS = B // NCORES            # batch rows per core
SPB = 64                    # timesteps per block
COLS = SPB * BS             # sbuf columns per block

# bias column registry in the packed [128, NBIAS] f32 bias tensor
_BC = {}
_nb = 0
for _name, _n in [('we_b', 2), ('wc_b', 1), ('wz_b', 2), ('wr_b', 2), ('wh_b', 2),
                  ('phi_b', 5), ('dck_b', 1), ('sck_b', 1), ('mck_b', 1),
                  ('se_b', 2), ('me_b', 2), ('dc2_b', 1), ('de2_b', 2),
                  ('p1_b', 1), ('oh_b', 1), ('p2_b', 1)]:
    _BC[_name] = _nb
    _nb += _n
NBIAS = _nb

# weight param shapes (bf16 lhsT tile grids)
WSHAPES = {
    'we':    [64, 2, 128],
    'wc':    [128, 2, 128],
    'wzx':   [128, 2, 2, 128], 'wrx': [128, 2, 2, 128], 'whx': [128, 2, 2, 128],
    'wzh':   [128, 2, 2, 128], 'wrh': [128, 2, 2, 128], 'whh': [128, 2, 2, 128],
    'wp':    [128, 2, 2, 128],
    'phism': [128, 3, 5, 128],
    'phid':  [128, 2, 5, 128],
    'dck':   [128, 5, 128], 'sck': [128, 5, 128], 'mck': [128, 5, 128],
    'sew':   [128, 2, 128], 'mew': [128, 2, 128], 'dew': [128, 2, 128],
    'dc2':   [128, 6, 128],
    'de2':   [128, 2, 128],
    'ohw':   [128, 2, 64],
    'p1w':   [128, 4, 128],
    'p2w':   [128, 64],
}


def build(T_=T):
    NBLK = T_ // SPB
    nc = bacc.Bacc("TRN2", target_bir_lowering=False, debug=False, num_devices=NCORES)

    obsT = nc.declare_dram_parameter("obsT", [D_S, (T_ + 2 * SPB) * BS], BF, isOutput=False)
    wdr = {k: nc.declare_dram_parameter(k, sh, BF, isOutput=False) for k, sh in WSHAPES.items()}
    bdr = nc.declare_dram_parameter("biases", [128, NBIAS], F32, isOutput=False)
    idr = nc.declare_dram_parameter("ident", [128, 128], BF, isOutput=False)
    # brows: K-row biases for PE-side bias injection: [which(0=De_b,1=Wp_b), 2, 128]
    brdr = nc.declare_dram_parameter("brows", [2, 2, 128], BF, isOutput=False)
    seldr = nc.declare_dram_parameter("sel", [2, 16], BF, isOutput=False)
    actT = nc.declare_dram_parameter("actT", [D_O, T_ * BS], F32, isOutput=True)
    predT = nc.declare_dram_parameter("predT", [D_S, T_ * BS], F32, isOutput=True)
    finals = nc.declare_dram_parameter("finals", [128, 48], F32, isOutput=True)

    with tile.TileContext(nc) as tc:
        with (
            tc.tile_pool(name="w", bufs=1) as wp,
            tc.tile_pool(name="st", bufs=1) as sp,
            tc.tile_pool(name="kp", bufs=3) as kp,
            tc.tile_pool(name="pss", bufs=2, space="PSUM") as pss,
            tc.tile_pool(name="psb", bufs=2, space="PSUM") as psb,
        ):
            # ---- resident weights & biases
            W = {}
            for k, sh in WSHAPES.items():
                W[k] = wp.tile(sh, BF, tag=k, name=f'w_{k}')
                nc.sync.dma_start(W[k][...], wdr[k][...])
            bia = wp.tile([128, NBIAS], F32, tag='bias')
            nc.sync.dma_start(bia[...], bdr[...])
            ident = wp.tile([128, 128], BF, tag='ident')
            nc.sync.dma_start(ident[...], idr[...])
            brows = wp.tile([2, 2, 128], BF, tag='brows')
            nc.sync.dma_start(brows[...], brdr[...])
            # selector rhs for K=2 bias matmuls: sel[k, c] = 1 iff c//8 == k
            sel = wp.tile([2, 16], BF, tag='sel')
            nc.sync.dma_start(sel[...], seldr[...])

            def bc(name, i=0):
                return bia[:, _BC[name] + i: _BC[name] + i + 1]

            def bc64(name):
                return bia[0:64, _BC[name]: _BC[name] + 1]

            # ---- persistent block buffers
            obs_sb = sp.tile([D_S, COLS], BF, tag='obs')
            sen = sp.tile([128, 2, SPB, BS], BF, tag='sen')
            xzr = sp.tile([128, SPB, 32], BF, tag='xzr')
            xh = sp.tile([128, SPB, 16], BF, tag='xh')
            slow_c = sp.tile([128, SPB, BS], BF, tag='slowc')
            m_c = sp.tile([128, SPB, 16], BF, tag='mc')
            psm = sp.tile([128, SPB, 40], BF, tag='psm')
            gch = sp.tile([128, SPB, 40], BF, tag='gch')
            # comb: prev-block [S_low | M] in cols 0:24; D-state ring in cols 24:40
            # (slot s holds D(s-1); slot SPB holds D(SPB-1) for the wrap copy)
            comb = sp.tile([128, SPB + 1, 40], BF, tag='comb')
            h_car = sp.tile([128, 16], BF, tag='hcar')
            snch = sp.tile([128, 2, SPB, BS], BF, tag='snch')
            mnch = sp.tile([128, 2, SPB, BS], BF, tag='mnch')

            def r2(ap, m=2):
                return ap.rearrange("p (m c) -> p m c", m=m)

            # ========== per-step pieces ==========
            def h_prev_ap(s):
                return h_car[:, :] if s % CHUNK == 0 else m_c[:, s - 1]

            def d_prev_ap(s):
                return comb[:, s, 24:40]

            # --- PE emissions (order matters: these define the PE queue) ---
            def pe_phid(s, p_ps):
                dp = d_prev_ap(s)
                for m in range(5):
                    for k in range(2):
                        nc.tensor.matmul(p_ps[:, m], W['phid'][:, k, m, :],
                                         dp[:, 8 * k: 8 * k + 8], start=False,
                                         stop=(m == 4 and k == 1))

            def pe_zr(s, zr_ps):
                hp = h_prev_ap(s)
                for gi, wname in ((0, 'wzh'), (1, 'wrh')):
                    for m in range(2):
                        for k in range(2):
                            nc.tensor.matmul(zr_ps[:, 2 * gi + m], W[wname][:, k, m, :],
                                             hp[:, 8 * k: 8 * k + 8],
                                             start=False,
                                             stop=(gi == 1 and m == 1 and k == 1))

            def pe_dc(s):
                a_ps = pss.tile([128, BS], F32, tag='stepI', name='a_ps')
                for k in range(5):
                    nc.tensor.matmul(a_ps[...], W['dck'][:, k, :], gch[:, s, 8 * k: 8 * k + 8],
                                     start=(k == 0), stop=(k == 4))
                return a_ps

            def pe_hc(s, rh, hc_ps):
                for m in range(2):
                    for k in range(2):
                        nc.tensor.matmul(hc_ps[:, m], W['whh'][:, k, m, :],
                                         rh[:, 8 * k: 8 * k + 8], start=False,
                                         stop=(m == 1 and k == 1))

            def pe_de(s, a_bf, d_ps):
                for m in range(2):
                    nc.tensor.matmul(d_ps[:, m], W['dew'][:, m, :], a_bf[...],
                                     start=False, stop=True)

            def pe_prop(s, hp_ps):
                for m in range(2):
                    for k in range(2):
                        nc.tensor.matmul(hp_ps[:, m], W['wp'][:, k, m, :],
                                         m_c[:, s, 8 * k: 8 * k + 8],
                                         start=False, stop=(m == 1 and k == 1))

            # ========== fused s-loop (GRU step s of block j; interaction step s of j-1) ==========
            def fused_steps(with_int=True, with_gru=True, s_lo=0, s_hi=SPB, fillers=None):
                fq = list(fillers) if fillers else []
                for s in range(s_lo, s_hi):
                    if fq and s >= SPB // 2 + 2:
                        fq.pop(0)()
                    prop = with_gru and (s % CHUNK == CHUNK - 1)
                    # --- early PSUM injections (no chain deps): streams + biases
                    if with_int:
                        p_ps = pss.tile([128, 5, BS], F32, tag='stepP', name='p_ps')
                        nc.tensor.matmul(p_ps[...], ident[...], psm[:, s],
                                         start=True, stop=False)
                        d_ps = pss.tile([128, 2, BS], F32, tag='stepI', name='d_ps')
                        nc.tensor.matmul(d_ps[...], brows[:, 0], sel[...],
                                         start=True, stop=False)
                    if with_gru:
                        zr_ps = pss.tile([128, 4, BS], F32, tag='stepG', name='zr_ps')
                        nc.tensor.matmul(zr_ps[...], ident[...], xzr[:, s],
                                         start=True, stop=False)
                        hc_ps = pss.tile([128, 2, BS], F32, tag='stepG', name='hc_ps')
                        nc.tensor.matmul(hc_ps[...], ident[...], xh[:, s],
                                         start=True, stop=False)
                        if prop:
                            hp_ps = pss.tile([128, 2, BS], F32, tag='stepG', name='hp_ps')
                            nc.tensor.matmul(hp_ps[...], brows[:, 1], sel[...],
                                             start=True, stop=False)
                    # --- recurrent matmul groups
                    if with_int:
                        pe_phid(s, p_ps)
                    if with_gru:
                        pe_zr(s, zr_ps)
                    # --- sigmoids straight from PSUM
                    if with_int:
                        sigp = kp.tile([128, 5, BS], F32, tag='sigp', name='sigp')
                        nc.scalar.activation(sigp[...], p_ps[...], AF.Sigmoid)
                        nc.vector.tensor_mul(r2(gch[:, s], 5), sigp[...], r2(comb[:, s], 5))
                    if with_gru:
                        sig = kp.tile([128, 4, BS], F32, tag='sigzr', name='sig')
                        nc.scalar.activation(sig[...], zr_ps[...], AF.Sigmoid)
                        hp = h_prev_ap(s)
                        rh = kp.tile([128, 16], BF, tag='rh', name='rh')
                        nc.vector.tensor_mul(r2(rh), sig[:, 2:4], r2(hp))
                    # --- second matmul wave
                    a_ps = pe_dc(s) if with_int else None
                    if with_gru:
                        pe_hc(s, rh, hc_ps)
                    if with_int:
                        a_bf = kp.tile([128, BS], BF, tag='abf', name='a_bf')
                        nc.scalar.activation(a_bf[...], a_ps[...], AF.Tanh, bias=bc('dck_b'))
                    if with_gru:
                        # t4 = (z - 1) * h  (overlaps hc matmuls)
                        t4 = kp.tile([128, 2, BS], F32, tag='gtmp', name='t4')
                        nc.vector.scalar_tensor_tensor(
                            t4[...], sig[:, 0:2], 1.0, r2(hp),
                            mybir.AluOpType.subtract, mybir.AluOpType.mult)
                    # --- final matmul + activations
                    if with_int:
                        pe_de(s, a_bf, d_ps)
                        nc.scalar.activation(r2(comb[:, s + 1, 24:40]), d_ps[...], AF.Tanh)
                    if with_gru:
                        hc = kp.tile([128, 2, BS], F32, tag='hc', name='hc')
                        nc.scalar.activation(hc[...], hc_ps[...], AF.Tanh)
                        t3 = kp.tile([128, 2, BS], F32, tag='gtmp', name='t3')
                        nc.vector.tensor_mul(t3[...], sig[:, 0:2], hc[...])
                        nc.vector.tensor_sub(r2(m_c[:, s]), t3[...], t4[...])
                        if prop:
                            pe_prop(s, hp_ps)
                            nc.scalar.activation(r2(h_car[:, :]), hp_ps[...], AF.Tanh)

            # ========== stage C piece: one P_SM matmul group ==========
            def c_group(m, lo=0, hi=SPB):
                ps = psb.tile([128, hi - lo, BS], F32, tag='blk', name='c_ps')
                nc.tensor.matmul(ps[...], W['phism'][:, 0, m, :], slow_c[:, lo:hi], start=True, stop=False)
                for kk in (1, 2):
                    nc.tensor.matmul(ps[...], W['phism'][:, kk, m, :],
                                     m_c[:, lo:hi, 8 * (kk - 1): 8 * kk],
                                     start=False, stop=(kk == 2))
                nc.vector.tensor_scalar_add(psm[:, lo:hi, 8 * m: 8 * m + 8], ps[...], bc('phi_b', m))

            # ========== stage A pieces: encode + GRU input precompute ==========
            def a_sen():
                for m in range(2):
                    ps = psb.tile([128, SPB, BS], F32, tag='blk', name='a_ps')
                    nc.tensor.matmul(ps[...], W['we'][:, m, :], obs_sb[...], start=True, stop=True)
                    nc.scalar.activation(sen[:, m], ps[...], AF.Tanh, bias=bc('we_b', m))

            def a_x(wname, dst, base, bname):
                for m in range(2):
                    ps = psb.tile([128, SPB, BS], F32, tag='blk', name='a_ps2')
                    for k in range(2):
                        nc.tensor.matmul(ps[...], W[wname][:, k, m, :], sen[:, k],
                                         start=(k == 0), stop=(k == 1))
                    nc.vector.tensor_scalar_add(
                        dst[:, :, base + 8 * m: base + 8 * (m + 1)], ps[...], bc(bname, m))

            def a_wc():
                ps = psb.tile([128, SPB, BS], F32, tag='blk', name='a_ps3')
                for k in range(2):
                    nc.tensor.matmul(ps[...], W['wc'][:, k, :], sen[:, k],
                                     start=(k == 0), stop=(k == 1))
                nc.scalar.activation(slow_c[...], ps[...], AF.Tanh, bias=bc('wc_b'))

            # ========== stage E pieces: deferred Sn/Mn + heads ==========
            def e_cmm(cw, cb, lo=0, hi=SPB):
                ps = psb.tile([128, hi - lo, BS], F32, tag='blk', name='e_ps')
                for k in range(5):
                    nc.tensor.matmul(ps[...], W[cw][:, k, :], gch[:, lo:hi, 8 * k: 8 * k + 8],
                                     start=(k == 0), stop=(k == 4))
                t1 = kp.tile([128, hi - lo, BS], BF, tag='t1', name='t1')
                nc.scalar.activation(t1[...], ps[...], AF.Tanh, bias=bc(cb))
                return t1

            def e_emm(ew, eb, t1, dst, lo=0, hi=SPB):
                for m in range(2):
                    ps2 = psb.tile([128, hi - lo, BS], F32, tag='blk', name='e_ps2')
                    nc.tensor.matmul(ps2[...], W[ew][:, m, :], t1[...], start=True, stop=True)
                    nc.scalar.activation(dst[:, m, lo:hi], ps2[...], AF.Tanh, bias=bc(eb, m))

            def rhs6(lo=0, hi=SPB):
                return [snch[:, 0, lo:hi], snch[:, 1, lo:hi], mnch[:, 0, lo:hi], mnch[:, 1, lo:hi],
                        comb[:, 1 + lo:1 + hi, 24:32], comb[:, 1 + lo:1 + hi, 32:40]]

            def e_dc2(lo=0, hi=SPB):
                ps = psb.tile([128, hi - lo, BS], F32, tag='blk', name='e_ps3')
                for k, r in enumerate(rhs6(lo, hi)):
                    nc.tensor.matmul(ps[...], W['dc2'][:, k, :], r,
                                     start=(k == 0), stop=(k == 5))
                t1 = kp.tile([128, hi - lo, BS], BF, tag='t1', name='t1b')
                nc.scalar.activation(t1[...], ps[...], AF.Tanh, bias=bc('dc2_b'))
                return t1

            def e_de2(t1, lo=0, hi=SPB):
                dec = kp.tile([128, 2, hi - lo, BS], BF, tag='dec', name='dec')
                for m in range(2):
                    ps2 = psb.tile([128, hi - lo, BS], F32, tag='blk', name='e_ps4')
                    nc.tensor.matmul(ps2[...], W['de2'][:, m, :], t1[...], start=True, stop=True)
                    nc.scalar.activation(dec[:, m], ps2[...], AF.Tanh, bias=bc('de2_b', m))
                return dec

            def e_oh(dec, out_off, lo=0, hi=SPB):
                ps3 = psb.tile([64, hi - lo, BS], F32, tag='blk', name='e_ps5')
                for k in range(2):
                    nc.tensor.matmul(ps3[...], W['ohw'][:, k, :], dec[:, k],
                                     start=(k == 0), stop=(k == 1))
                act_o = kp.tile([64, hi - lo, BS], F32, tag='acto', name='act_o')
                nc.vector.tensor_scalar_add(act_o[...], ps3[...], bc64('oh_b'))
                nc.sync.dma_start(actT[:, ds(out_off + lo * BS, (hi - lo) * BS)], act_o[...])

            def e_p1(lo=0, hi=SPB):
                ps = psb.tile([128, hi - lo, BS], F32, tag='blk', name='e_ps6')
                for k, r in enumerate(rhs6(lo, hi)[:4]):
                    nc.tensor.matmul(ps[...], W['p1w'][:, k, :], r,
                                     start=(k == 0), stop=(k == 3))
                t1 = kp.tile([128, hi - lo, BS], BF, tag='t1', name='t1c')
                nc.scalar.activation(t1[...], ps[...], AF.Tanh, bias=bc('p1_b'))
                return t1

            def e_p2(t1, out_off, lo=0, hi=SPB):
                ps3 = psb.tile([64, hi - lo, BS], F32, tag='blk', name='e_ps7')
                nc.tensor.matmul(ps3[...], W['p2w'][...], t1[...], start=True, stop=True)
                pred_o = kp.tile([64, hi - lo, BS], F32, tag='predo', name='pred_o')
                nc.vector.tensor_scalar_add(pred_o[...], ps3[...], bc64('p2_b'))
                nc.sync.dma_start(predT[:, ds(out_off + lo * BS, (hi - lo) * BS)], pred_o[...])

            def fill_e1(out_off):
                HB = SPB // 2
                t1s = e_cmm('sck', 'sck_b', 0, HB)
                t1m = e_cmm('mck', 'mck_b', 0, HB)
                e_emm('sew', 'se_b', t1s, snch, 0, HB)
                e_emm('mew', 'me_b', t1m, mnch, 0, HB)
                t1d = e_dc2(0, HB)
                dec = e_de2(t1d, 0, HB)
                e_oh(dec, out_off, 0, HB)
                t1p = e_p1(0, HB)
                e_p2(t1p, out_off, 0, HB)

            def stage_EA2(out_off):
                HB = SPB // 2
                a_sen()
                t1s = e_cmm('sck', 'sck_b', HB, SPB)
                a_x('wzx', xzr, 0, 'wz_b')
                t1m = e_cmm('mck', 'mck_b', HB, SPB)
                e_emm('sew', 'se_b', t1s, snch, HB, SPB)
                e_emm('mew', 'me_b', t1m, mnch, HB, SPB)
                a_x('wrx', xzr, 16, 'wr_b')
                t1d = e_dc2(HB, SPB)
                a_x('whx', xh, 0, 'wh_b')
                dec = e_de2(t1d, HB, SPB)
                e_oh(dec, out_off, HB, SPB)
                t1p = e_p1(HB, SPB)
                a_wc()
                e_p2(t1p, out_off, HB, SPB)

            def stage_CEA(out_off, with_e=True, with_a=True, with_c=True):
                """Interleaved tail: C groups fill E's activation waits, A fills the rest."""
                if with_c:
                    for m in range(3):
                        c_group(m)
                if with_e:
                    t1s = e_cmm('sck', 'sck_b')
                if with_c:
                    c_group(3)
                if with_e:
                    t1m = e_cmm('mck', 'mck_b')
                if with_c:
                    c_group(4)
                    copies()
                if with_a:
                    a_sen()
                if with_e:
                    e_emm('sew', 'se_b', t1s, snch)
                    e_emm('mew', 'me_b', t1m, mnch)
                if with_a:
                    a_x('wzx', xzr, 0, 'wz_b')
                if with_e:
                    t1d = e_dc2()
                if with_a:
                    a_x('wrx', xzr, 16, 'wr_b')
                if with_e:
                    dec = e_de2(t1d)
                if with_a:
                    a_x('whx', xh, 0, 'wh_b')
                if with_e:
                    e_oh(dec, out_off)
                    t1p = e_p1()
                if with_a:
                    a_wc()
                if with_e:
                    e_p2(t1p, out_off)

            def copies():
                nc.vector.tensor_copy(comb[:, 0:SPB, 0:8], slow_c[...])
                nc.vector.tensor_copy(comb[:, 0:SPB, 8:24], m_c[...])
                nc.vector.tensor_copy(comb[:, 0:1, 24:40], comb[:, SPB:SPB + 1, 24:40])

            # ================= prologue: block 0 (+ A(1)) =================
            nc.vector.memset(h_car[...], 0.0)
            nc.vector.memset(comb[...], 0.0)
            nc.sync.dma_start(obs_sb[...], obsT[:, 0:COLS])
            a_sen()
            a_x('wzx', xzr, 0, 'wz_b')
            a_x('wrx', xzr, 16, 'wr_b')
            a_x('whx', xh, 0, 'wh_b')
            a_wc()                         # A(0)
            nc.sync.dma_start(obs_sb[...], obsT[:, COLS: 2 * COLS])
            fused_steps(with_int=False)    # B(0)
            stage_CEA(0, with_e=False)     # C(0) + copies + A(1)

            # ================= main loop: j in [1, NBLK) =================
            with tc.For_i(1, NBLK, 1,
                          hint_engines=(mybir.EngineType.PE, mybir.EngineType.DVE,
                                        mybir.EngineType.Activation)) as j:
                nc.sync.dma_start(obs_sb[...], obsT[:, ds((j + 1) * COLS, COLS)])
                HB = SPB // 2
                off = (j - 1) * COLS
                fillers = []
                for _m in range(5):
                    fillers.append(lambda m=_m: c_group(m, 0, HB))
                fillers.append(lambda: fill_e1(off))
                fused_steps(fillers=fillers)         # B(j) + D(j-1) + C/E first halves
                for _m in range(5):
                    c_group(_m, HB, SPB)
                copies()
                stage_EA2(off)                       # E-h2(j-1) + A(j+1)

            # ================= epilogue: interaction + heads for last block =================
            fused_steps(with_gru=False)    # D(NBLK-1)
            t1s = e_cmm('sck', 'sck_b')
            t1m = e_cmm('mck', 'mck_b')
            e_emm('sew', 'se_b', t1s, snch)
            e_emm('mew', 'me_b', t1m, mnch)
            t1d = e_dc2()
            dec = e_de2(t1d)
            e_oh(dec, (NBLK - 1) * COLS)
            t1p = e_p1()
            e_p2(t1p, (NBLK - 1) * COLS)
            nc.gpsimd.dma_start(finals[:, 0:8], snch[:, 0, SPB - 1])
            nc.gpsimd.dma_start(finals[:, 8:16], snch[:, 1, SPB - 1])
            nc.gpsimd.dma_start(finals[:, 16:24], mnch[:, 0, SPB - 1])
            nc.gpsimd.dma_start(finals[:, 24:32], mnch[:, 1, SPB - 1])
            nc.gpsimd.dma_start(finals[:, 32:40], comb[:, SPB, 24:32])
            nc.gpsimd.dma_start(finals[:, 40:48], comb[:, SPB, 32:40])

    nc.compile()
    return nc


# ======================= host-side packing =======================

def _tiles(Wm, kt, mt, mw=128):
    """[K, M] f32 -> [128, kt, mt, mw] bf16 lhsT tile grid (squeezed per caller)."""
    out = np.zeros((128, kt, mt, mw), np.float32)
    for k in range(kt):
        for m in range(mt):
            blk = Wm[k * 128:(k + 1) * 128, m * mw:(m + 1) * mw]
            out[:blk.shape[0], k, m, :blk.shape[1]] = blk
    return out.astype(BF16)


def _pack_weights(p):
    KEEP = np.r_[0:128, 256:768]
    phi = p['phi']
    w = {}
    w['we'] = _tiles(p['We'], 1, 2)[0:64, 0]                      # [64,2,128]
    w['wc'] = _tiles(p['Wc'], 2, 1)[:, :, 0]                      # [128,2,128]
    for nm, mat in (('wzx', p['Wz'][:D_M]), ('wrx', p['Wr'][:D_M]), ('whx', p['Wh'][:D_M]),
                    ('wzh', p['Wz'][D_M:]), ('wrh', p['Wr'][D_M:]), ('whh', p['Wh'][D_M:]),
                    ('wp', p['Wp'])):
        w[nm] = _tiles(mat, 2, 2)
    phism = np.concatenate([phi[0:128], phi[256:512]], axis=0)[:, KEEP]   # [384, 640]
    w['phism'] = _tiles(phism, 3, 5)
    w['phid'] = _tiles(phi[512:768][:, KEEP], 2, 5)
    w['dck'] = _tiles(p['Dc'][KEEP], 5, 1)[:, :, 0]
    w['sck'] = _tiles(p['Sc'][KEEP], 5, 1)[:, :, 0]
    w['mck'] = _tiles(p['Mc'][KEEP], 5, 1)[:, :, 0]
    w['sew'] = _tiles(p['Se'], 1, 2)[:, 0]
    w['mew'] = _tiles(p['Me'], 1, 2)[:, 0]
    w['dew'] = _tiles(p['De'], 1, 2)[:, 0]
    w['dc2'] = _tiles(p['dc'], 6, 1)[:, :, 0]
    w['de2'] = _tiles(p['de'], 1, 2)[:, 0]
    w['ohw'] = _tiles(p['oh'], 2, 1, mw=64)[:, :, 0]
    w['p1w'] = _tiles(p['p1'], 4, 1)[:, :, 0]
    w['p2w'] = _tiles(p['p2'], 1, 1, mw=64)[:, 0, 0]
    for k, sh in WSHAPES.items():
        assert list(w[k].shape) == sh, (k, w[k].shape, sh)
    return w


def _pack_biases(p):
    KEEP = np.r_[0:128, 256:768]
    bia = np.zeros((128, NBIAS), np.float32)

    def put(name, vec):
        vec = np.asarray(vec, np.float32)
        n = (len(vec) + 127) // 128
        for i in range(n):
            seg = vec[i * 128:(i + 1) * 128]
            bia[:len(seg), _BC[name] + i] = seg
    put('we_b', p['We_b']); put('wc_b', p['Wc_b'])
    put('wz_b', p['Wz_b']); put('wr_b', p['Wr_b']); put('wh_b', p['Wh_b'])
    put('phi_b', np.asarray(p['phi_b'])[KEEP])
    put('dck_b', p['Dc_b']); put('sck_b', p['Sc_b']); put('mck_b', p['Mc_b'])
    put('se_b', p['Se_b']); put('me_b', p['Me_b'])
    put('dc2_b', p['dc_b']); put('de2_b', p['de_b'])
    put('p1_b', p['p1_b']); put('oh_b', p['oh_b']); put('p2_b', p['p2_b'])
    return bia


def _pack_brows(p):
    # [k_row(=m tile), which(0=De_b,1=Wp_b), 128]: lhsT slice brows[:, w] is [2, 128]
    br = np.zeros((2, 2, 128), np.float32)
    for m in range(2):
        br[m, 0] = np.asarray(p['De_b'], np.float32)[m * 128:(m + 1) * 128]
        br[m, 1] = np.asarray(p['Wp_b'], np.float32)[m * 128:(m + 1) * 128]
    return br.astype(BF16)


_NC_CACHE = {}


def _get_nc(T_):
    if T_ not in _NC_CACHE:
        _NC_CACHE[T_] = build(T_)
    return _NC_CACHE[T_]


def kernel(obs_seq, params):
    obs = np.asarray(obs_seq, np.float32)
    p = {k: np.asarray(v, np.float32) for k, v in params.items()}
    Bf, Tf = obs.shape[0], obs.shape[1]
    nc = _get_nc(Tf)

    common = _pack_weights(p)
    common['biases'] = _pack_biases(p)
    common['ident'] = np.eye(128, dtype=np.float32).astype(BF16)
    selm = np.zeros((2, 16), np.float32)
    selm[0, 0:8] = 1.0
    selm[1, 8:16] = 1.0
    common['sel'] = selm.astype(BF16)
    common['brows'] = _pack_brows(p)

    in_maps = []
    for c in range(NCORES):
        shard = obs[c * BS:(c + 1) * BS]                      # [BS, T, D_S]
        obsT = shard.transpose(2, 1, 0).reshape(D_S, Tf * BS)  # col = t*BS + b
        pad = np.zeros((D_S, 2 * SPB * BS), np.float32)
        obsT = np.concatenate([obsT, pad], axis=1).astype(BF16)
        im = dict(common)
        im['obsT'] = obsT
        in_maps.append(im)

    res = run_bass_kernel_spmd(nc, in_maps, core_ids=list(range(NCORES)))

    actions = np.zeros((Bf, Tf, D_O), np.float32)
    preds = np.zeros((Bf, Tf, D_S), np.float32)
    S_last = np.zeros((Bf, D_ST), np.float32)
    M_last = np.zeros((Bf, D_ST), np.float32)
    D_last = np.zeros((Bf, D_ST), np.float32)
    for c in range(NCORES):
        r = res.results[c]
        sl = slice(c * BS, (c + 1) * BS)
        actions[sl] = r['actT'].reshape(D_O, Tf, BS).transpose(2, 1, 0)
        preds[sl] = r['predT'].reshape(D_S, Tf, BS).transpose(2, 1, 0)
        fin = r['finals']                                     # [128, 48]
        S_last[sl] = fin[:, 0:16].reshape(128, 2, BS).transpose(2, 1, 0).reshape(BS, 256)
        M_last[sl] = fin[:, 16:32].reshape(128, 2, BS).transpose(2, 1, 0).reshape(BS, 256)
        D_last[sl] = fin[:, 32:48].reshape(128, 2, BS).transpose(2, 1, 0).reshape(BS, 256)
    return actions, preds, S_last, M_last, D_last


# revision 19
# speedup vs baseline: 1.0188x; 1.0001x over previous
"""Trainium2 Bass kernel: fused GRU + interaction double-recurrence (ANIMA).

Strategy: data-parallel over batch (8 rows/core on 8 cores). Everything runs in
transposed [feature, batch] layout (feature on partitions). One main For_i loop
over blocks of 32 timesteps fuses all phases; per-step emission is ordered so
the interaction chain (the critical path: De -> tanh -> phi_D -> sigmoid -> Dc
-> tanh -> De) is never queued behind non-critical work on PE/ACT/DVE, and the
GRU + encode/head matmuls fill its stall windows.
  body j: [obs dma j+1] [s-loop: GRU(j) + interaction(j-1) interleaved]
          [C: P_SM(j)] [E: Sn/Mn + heads (j-1)] [A: encode/precompute (j+1)]
Intermediates never round-trip through DRAM; DMA is only obs in / outputs out.
"""
import sys
sys.path.insert(0, '/opt/trn_rl_repo')
import numpy as np
import ml_dtypes

import concourse.bass as bass
import concourse.bacc as bacc
import concourse.mybir as mybir
from concourse import tile
from concourse.bass_utils import run_bass_kernel_spmd

BF16 = ml_dtypes.bfloat16
F32 = mybir.dt.float32
BF = mybir.dt.bfloat16
AF = mybir.ActivationFunctionType
ds = bass.ds

B, T, D_S, D_M, D_B, D_ST, D_O, CHUNK = 64, 2048, 64, 256, 128, 256, 64, 16
NCORES = 8
BS = B // NCORES            # batch rows per core
SPB = 64                    # timesteps per block
COLS = SPB * BS             # sbuf columns per block

# bias column registry in the packed [128, NBIAS] f32 bias tensor
_BC = {}
_nb = 0
for _name, _n in [('we_b', 2), ('wc_b', 1), ('wz_b', 2), ('wr_b', 2), ('wh_b', 2),
                  ('phi_b', 5), ('dck_b', 1), ('sck_b', 1), ('mck_b', 1),
                  ('se_b', 2), ('me_b', 2), ('dc2_b', 1), ('de2_b', 2),
                  ('p1_b', 1), ('oh_b', 1), ('p2_b', 1)]:
    _BC[_name] = _nb
    _nb += _n
NBIAS = _nb

# weight param shapes (bf16 lhsT tile grids)
WSHAPES = {
    'we':    [64, 2, 128],
    'wc':    [128, 2, 128],
    'wzx':   [128, 2, 2, 128], 'wrx': [128, 2, 2, 128], 'whx': [128, 2, 2, 128],
    'wzh':   [128, 2, 2, 128], 'wrh': [128, 2, 2, 128], 'whh': [128, 2, 2, 128],
    'wp':    [128, 2, 2, 128],
    'phism': [128, 3, 5, 128],
    'phid':  [128, 2, 5, 128],
    'dck':   [128, 5, 128], 'sck': [128, 5, 128], 'mck': [128, 5, 128],
    'sew':   [128, 2, 128], 'mew': [128, 2, 128], 'dew': [128, 2, 128],
    'dc2':   [128, 6, 128],
    'de2':   [128, 2, 128],
    'ohw':   [128, 2, 64],
    'p1w':   [128, 4, 128],
    'p2w':   [128, 64],
}


def build(T_=T):
    NBLK = T_ // SPB
    nc = bacc.Bacc("TRN2", target_bir_lowering=False, debug=False, num_devices=NCORES)

    obsT = nc.declare_dram_parameter("obsT", [D_S, (T_ + 2 * SPB) * BS], BF, isOutput=False)
    wdr = {k: nc.declare_dram_parameter(k, sh, BF, isOutput=False) for k, sh in WSHAPES.items()}
    bdr = nc.declare_dram_parameter("biases", [128, NBIAS], F32, isOutput=False)
    idr = nc.declare_dram_parameter("ident", [128, 128], BF, isOutput=False)
    # brows: K-row biases for PE-side bias injection: [which(0=De_b,1=Wp_b), 2, 128]
    brdr = nc.declare_dram_parameter("brows", [2, 2, 128], BF, isOutput=False)
    seldr = nc.declare_dram_parameter("sel", [2, 16], BF, isOutput=False)
    actT = nc.declare_dram_parameter("actT", [D_O, T_ * BS], F32, isOutput=True)
    predT = nc.declare_dram_parameter("predT", [D_S, T_ * BS], F32, isOutput=True)
    finals = nc.declare_dram_parameter("finals", [128, 48], F32, isOutput=True)

    with tile.TileContext(nc) as tc:
        with (
            tc.tile_pool(name="w", bufs=1) as wp,
            tc.tile_pool(name="st", bufs=1) as sp,
            tc.tile_pool(name="kp", bufs=3) as kp,
            tc.tile_pool(name="pss", bufs=2, space="PSUM") as pss,
            tc.tile_pool(name="psb", bufs=2, space="PSUM") as psb,
        ):
            # ---- resident weights & biases
            W = {}
            for k, sh in WSHAPES.items():
                W[k] = wp.tile(sh, BF, tag=k, name=f'w_{k}')
                nc.sync.dma_start(W[k][...], wdr[k][...])
            bia = wp.tile([128, NBIAS], F32, tag='bias')
            nc.sync.dma_start(bia[...], bdr[...])
            ident = wp.tile([128, 128], BF, tag='ident')
            nc.sync.dma_start(ident[...], idr[...])
            brows = wp.tile([2, 2, 128], BF, tag='brows')
            nc.sync.dma_start(brows[...], brdr[...])
            # selector rhs for K=2 bias matmuls: sel[k, c] = 1 iff c//8 == k
            sel = wp.tile([2, 16], BF, tag='sel')
            nc.sync.dma_start(sel[...], seldr[...])

            def bc(name, i=0):
                return bia[:, _BC[name] + i: _BC[name] + i + 1]

            def bc64(name):
                return bia[0:64, _BC[name]: _BC[name] + 1]

            # ---- persistent block buffers
            obs_sb = sp.tile([D_S, COLS], BF, tag='obs')
            sen = sp.tile([128, 2, SPB, BS], BF, tag='sen')
            xzr = sp.tile([128, SPB, 32], BF, tag='xzr')
            xh = sp.tile([128, SPB, 16], BF, tag='xh')
            slow_c = sp.tile([128, SPB, BS], BF, tag='slowc')
            m_c = sp.tile([128, SPB, 16], BF, tag='mc')
            psm = sp.tile([128, SPB, 40], BF, tag='psm')
            gch = sp.tile([128, SPB, 40], BF, tag='gch')
            # comb: prev-block [S_low | M] in cols 0:24; D-state ring in cols 24:40
            # (slot s holds D(s-1); slot SPB holds D(SPB-1) for the wrap copy)
            comb = sp.tile([128, SPB + 1, 40], BF, tag='comb')
            h_car = sp.tile([128, 16], BF, tag='hcar')
            snch = sp.tile([128, 2, SPB, BS], BF, tag='snch')
            mnch = sp.tile([128, 2, SPB, BS], BF, tag='mnch')

            def r2(ap, m=2):
                return ap.rearrange("p (m c) -> p m c", m=m)

            # ========== per-step pieces ==========
            def h_prev_ap(s):
                return h_car[:, :] if s % CHUNK == 0 else m_c[:, s - 1]

            def d_prev_ap(s):
                return comb[:, s, 24:40]

            # --- PE emissions (order matters: these define the PE queue) ---
            def pe_phid(s, p_ps):
                dp = d_prev_ap(s)
                for m in range(5):
                    for k in range(2):
                        nc.tensor.matmul(p_ps[:, m], W['phid'][:, k, m, :],
                                         dp[:, 8 * k: 8 * k + 8], start=False,
                                         stop=(m == 4 and k == 1))

            def pe_zr(s, zr_ps):
                hp = h_prev_ap(s)
                for gi, wname in ((0, 'wzh'), (1, 'wrh')):
                    for m in range(2):
                        for k in range(2):
                            nc.tensor.matmul(zr_ps[:, 2 * gi + m], W[wname][:, k, m, :],
                                             hp[:, 8 * k: 8 * k + 8],
                                             start=False,
                                             stop=(gi == 1 and m == 1 and k == 1))

            def pe_dc(s):
                a_ps = pss.tile([128, BS], F32, tag='stepI', name='a_ps')
                for k in range(5):
                    nc.tensor.matmul(a_ps[...], W['dck'][:, k, :], gch[:, s, 8 * k: 8 * k + 8],
                                     start=(k == 0), stop=(k == 4))
                return a_ps

            def pe_hc(s, rh, hc_ps):
                for m in range(2):
                    for k in range(2):
                        nc.tensor.matmul(hc_ps[:, m], W['whh'][:, k, m, :],
                                         rh[:, 8 * k: 8 * k + 8], start=False,
                                         stop=(m == 1 and k == 1))

            def pe_de(s, a_bf, d_ps):
                for m in range(2):
                    nc.tensor.matmul(d_ps[:, m], W['dew'][:, m, :], a_bf[...],
                                     start=False, stop=True)

            def pe_prop(s, hp_ps):
                for m in range(2):
                    for k in range(2):
                        nc.tensor.matmul(hp_ps[:, m], W['wp'][:, k, m, :],
                                         m_c[:, s, 8 * k: 8 * k + 8],
                                         start=False, stop=(m == 1 and k == 1))

            # ========== fused s-loop (GRU step s of block j; interaction step s of j-1) ==========
            def fused_steps(with_int=True, with_gru=True, s_lo=0, s_hi=SPB, fillers=None):
                fq = list(fillers) if fillers else []
                for s in range(s_lo, s_hi):
                    if fq and s >= SPB // 2 + 2:
                        fq.pop(0)()
                    prop = with_gru and (s % CHUNK == CHUNK - 1)
                    # --- early PSUM injections (no chain deps): streams + biases
                    if with_int:
                        p_ps = pss.tile([128, 5, BS], F32, tag='stepP', name='p_ps')
                        nc.tensor.matmul(p_ps[...], ident[...], psm[:, s],
                                         start=True, stop=False)
                        d_ps = pss.tile([128, 2, BS], F32, tag='stepI', name='d_ps')
                        nc.tensor.matmul(d_ps[...], brows[:, 0], sel[...],
                                         start=True, stop=False)
                    if with_gru:
                        zr_ps = pss.tile([128, 4, BS], F32, tag='stepG', name='zr_ps')
                        nc.tensor.matmul(zr_ps[...], ident[...], xzr[:, s],
                                         start=True, stop=False)
                        hc_ps = pss.tile([128, 2, BS], F32, tag='stepG', name='hc_ps')
                        nc.tensor.matmul(hc_ps[...], ident[...], xh[:, s],
                                         start=True, stop=False)
                        if prop:
                            hp_ps = pss.tile([128, 2, BS], F32, tag='stepG', name='hp_ps')
                            nc.tensor.matmul(hp_ps[...], brows[:, 1], sel[...],
                                             start=True, stop=False)
                    # --- recurrent matmul groups
                    if with_int:
                        pe_phid(s, p_ps)
                    if with_gru:
                        pe_zr(s, zr_ps)
                    # --- sigmoids straight from PSUM
                    if with_int:
                        sigp = kp.tile([128, 5, BS], F32, tag='sigp', name='sigp')
                        nc.scalar.activation(sigp[...], p_ps[...], AF.Sigmoid)
                        nc.vector.tensor_mul(r2(gch[:, s], 5), sigp[...], r2(comb[:, s], 5))
                    if with_gru:
                        sig = kp.tile([128, 4, BS], F32, tag='sigzr', name='sig')
                        nc.scalar.activation(sig[...], zr_ps[...], AF.Sigmoid)
                        hp = h_prev_ap(s)
                        rh = kp.tile([128, 16], BF, tag='rh', name='rh')
                        nc.vector.tensor_mul(r2(rh), sig[:, 2:4], r2(hp))
                    # --- second matmul wave
                    a_ps = pe_dc(s) if with_int else None
                    if with_gru:
                        pe_hc(s, rh, hc_ps)
                    if with_int:
                        a_bf = kp.tile([128, BS], BF, tag='abf', name='a_bf')
                        nc.scalar.activation(a_bf[...], a_ps[...], AF.Tanh, bias=bc('dck_b'))
                    if with_gru:
                        # t4 = (z - 1) * h  (overlaps hc matmuls)
                        t4 = kp.tile([128, 2, BS], F32, tag='gtmp', name='t4')
                        nc.vector.scalar_tensor_tensor(
                            t4[...], sig[:, 0:2], 1.0, r2(hp),
                            mybir.AluOpType.subtract, mybir.AluOpType.mult)
                    # --- final matmul + activations
                    if with_int:
                        pe_de(s, a_bf, d_ps)
                        nc.scalar.activation(r2(comb[:, s + 1, 24:40]), d_ps[...], AF.Tanh)
                    if with_gru:
                        hc = kp.tile([128, 2, BS], F32, tag='hc', name='hc')
                        nc.scalar.activation(hc[...], hc_ps[...], AF.Tanh)
                        t3 = kp.tile([128, 2, BS], F32, tag='gtmp', name='t3')
                        nc.vector.tensor_mul(t3[...], sig[:, 0:2], hc[...])
                        nc.vector.tensor_sub(r2(m_c[:, s]), t3[...], t4[...])
                        if prop:
                            pe_prop(s, hp_ps)
                            nc.scalar.activation(r2(h_car[:, :]), hp_ps[...], AF.Tanh)

            # ========== stage C piece: one P_SM matmul group ==========
            def c_group(m, lo=0, hi=SPB):
                ps = psb.tile([128, hi - lo, BS], F32, tag='blk', name='c_ps')
                nc.tensor.matmul(ps[...], W['phism'][:, 0, m, :], slow_c[:, lo:hi], start=True, stop=False)
                for kk in (1, 2):
                    nc.tensor.matmul(ps[...], W['phism'][:, kk, m, :],
                                     m_c[:, lo:hi, 8 * (kk - 1): 8 * kk],
                                     start=False, stop=(kk == 2))
                nc.vector.tensor_scalar_add(psm[:, lo:hi, 8 * m: 8 * m + 8], ps[...], bc('phi_b', m))

            # ========== stage A pieces: encode + GRU input precompute ==========
            def a_sen():
                for m in range(2):
                    ps = psb.tile([128, SPB, BS], F32, tag='blk', name='a_ps')
                    nc.tensor.matmul(ps[...], W['we'][:, m, :], obs_sb[...], start=True, stop=True)
                    nc.scalar.activation(sen[:, m], ps[...], AF.Tanh, bias=bc('we_b', m))

            def a_x(wname, dst, base, bname):
                for m in range(2):
                    ps = psb.tile([128, SPB, BS], F32, tag='blk', name='a_ps2')
                    for k in range(2):
                        nc.tensor.matmul(ps[...], W[wname][:, k, m, :], sen[:, k],
                                         start=(k == 0), stop=(k == 1))
                    nc.vector.tensor_scalar_add(
                        dst[:, :, base + 8 * m: base + 8 * (m + 1)], ps[...], bc(bname, m))

            def a_wc():
                ps = psb.tile([128, SPB, BS], F32, tag='blk', name='a_ps3')
                for k in range(2):
                    nc.tensor.matmul(ps[...], W['wc'][:, k, :], sen[:, k],
                                     start=(k == 0), stop=(k == 1))
                nc.scalar.activation(slow_c[...], ps[...], AF.Tanh, bias=bc('wc_b'))

            # ========== stage E pieces: deferred Sn/Mn + heads ==========
            def e_cmm(cw, cb, lo=0, hi=SPB):
                ps = psb.tile([128, hi - lo, BS], F32, tag='blk', name='e_ps')
                for k in range(5):
                    nc.tensor.matmul(ps[...], W[cw][:, k, :], gch[:, lo:hi, 8 * k: 8 * k + 8],
                                     start=(k == 0), stop=(k == 4))
                t1 = kp.tile([128, hi - lo, BS], BF, tag='t1', name='t1')
                nc.scalar.activation(t1[...], ps[...], AF.Tanh, bias=bc(cb))
                return t1

            def e_emm(ew, eb, t1, dst, lo=0, hi=SPB):
                for m in range(2):
                    ps2 = psb.tile([128, hi - lo, BS], F32, tag='blk', name='e_ps2')
                    nc.tensor.matmul(ps2[...], W[ew][:, m, :], t1[...], start=True, stop=True)
                    nc.scalar.activation(dst[:, m, lo:hi], ps2[...], AF.Tanh, bias=bc(eb, m))

            def rhs6(lo=0, hi=SPB):
                return [snch[:, 0, lo:hi], snch[:, 1, lo:hi], mnch[:, 0, lo:hi], mnch[:, 1, lo:hi],
                        comb[:, 1 + lo:1 + hi, 24:32], comb[:, 1 + lo:1 + hi, 32:40]]

            def e_dc2(lo=0, hi=SPB):
                ps = psb.tile([128, hi - lo, BS], F32, tag='blk', name='e_ps3')
                for k, r in enumerate(rhs6(lo, hi)):
                    nc.tensor.matmul(ps[...], W['dc2'][:, k, :], r,
                                     start=(k == 0), stop=(k == 5))
                t1 = kp.tile([128, hi - lo, BS], BF, tag='t1', name='t1b')
                nc.scalar.activation(t1[...], ps[...], AF.Tanh, bias=bc('dc2_b'))
                return t1

            def e_de2(t1, lo=0, hi=SPB):
                dec = kp.tile([128, 2, hi - lo, BS], BF, tag='dec', name='dec')
                for m in range(2):
                    ps2 = psb.tile([128, hi - lo, BS], F32, tag='blk', name='e_ps4')
                    nc.tensor.matmul(ps2[...], W['de2'][:, m, :], t1[...], start=True, stop=True)
                    nc.scalar.activation(dec[:, m], ps2[...], AF.Tanh, bias=bc('de2_b', m))
                return dec

            def e_oh(dec, out_off, lo=0, hi=SPB):
                ps3 = psb.tile([64, hi - lo, BS], F32, tag='blk', name='e_ps5')
                for k in range(2):
                    nc.tensor.matmul(ps3[...], W['ohw'][:, k, :], dec[:, k],
                                     start=(k == 0), stop=(k == 1))
                act_o = kp.tile([64, hi - lo, BS], F32, tag='acto', name='act_o')
                nc.vector.tensor_scalar_add(act_o[...], ps3[...], bc64('oh_b'))
                nc.sync.dma_start(actT[:, ds(out_off + lo * BS, (hi - lo) * BS)], act_o[...])

            def e_p1(lo=0, hi=SPB):
                ps = psb.tile([128, hi - lo, BS], F32, tag='blk', name='e_ps6')
                for k, r in enumerate(rhs6(lo, hi)[:4]):
                    nc.tensor.matmul(ps[...], W['p1w'][:, k, :], r,
                                     start=(k == 0), stop=(k == 3))
                t1 = kp.tile([128, hi - lo, BS], BF, tag='t1', name='t1c')
                nc.scalar.activation(t1[...], ps[...], AF.Tanh, bias=bc('p1_b'))
                return t1

            def e_p2(t1, out_off, lo=0, hi=SPB):
                ps3 = psb.tile([64, hi - lo, BS], F32, tag='blk', name='e_ps7')
                nc.tensor.matmul(ps3[...], W['p2w'][...], t1[...], start=True, stop=True)
                pred_o = kp.tile([64, hi - lo, BS], F32, tag='predo', name='pred_o')
                nc.vector.tensor_scalar_add(pred_o[...], ps3[...], bc64('p2_b'))
                nc.sync.dma_start(predT[:, ds(out_off + lo * BS, (hi - lo) * BS)], pred_o[...])

            def e1_fillers(out_off, lo, hi):
                st = {}
                return [
                    lambda: st.__setitem__('t1s', e_cmm('sck', 'sck_b', lo, hi)),
                    lambda: st.__setitem__('t1m', e_cmm('mck', 'mck_b', lo, hi)),
                    lambda: e_emm('sew', 'se_b', st['t1s'], snch, lo, hi),
                    lambda: e_emm('mew', 'me_b', st['t1m'], mnch, lo, hi),
                    lambda: st.__setitem__('t1d', e_dc2(lo, hi)),
                    lambda: st.__setitem__('dec', e_de2(st['t1d'], lo, hi)),
                    lambda: e_oh(st['dec'], out_off, lo, hi),
                    lambda: st.__setitem__('t1p', e_p1(lo, hi)),
                    lambda: e_p2(st['t1p'], out_off, lo, hi),
                ]

            def stage_EA2(out_off):
                HB = SPB // 2
                a_sen()
                t1s = e_cmm('sck', 'sck_b', HB, SPB)
                a_x('wzx', xzr, 0, 'wz_b')
                t1m = e_cmm('mck', 'mck_b', HB, SPB)
                e_emm('sew', 'se_b', t1s, snch, HB, SPB)
                e_emm('mew', 'me_b', t1m, mnch, HB, SPB)
                a_x('wrx', xzr, 16, 'wr_b')
                t1d = e_dc2(HB, SPB)
                a_x('whx', xh, 0, 'wh_b')
                dec = e_de2(t1d, HB, SPB)
                e_oh(dec, out_off, HB, SPB)
                t1p = e_p1(HB, SPB)
                a_wc()
                e_p2(t1p, out_off, HB, SPB)

            def stage_CEA(out_off, with_e=True, with_a=True, with_c=True):
                """Interleaved tail: C groups fill E's activation waits, A fills the rest."""
                if with_c:
                    for m in range(3):
                        c_group(m)
                if with_e:
                    t1s = e_cmm('sck', 'sck_b')
                if with_c:
                    c_group(3)
                if with_e:
                    t1m = e_cmm('mck', 'mck_b')
                if with_c:
                    c_group(4)
                    copies()
                if with_a:
                    a_sen()
                if with_e:
                    e_emm('sew', 'se_b', t1s, snch)
                    e_emm('mew', 'me_b', t1m, mnch)
                if with_a:
                    a_x('wzx', xzr, 0, 'wz_b')
                if with_e:
                    t1d = e_dc2()
                if with_a:
                    a_x('wrx', xzr, 16, 'wr_b')
                if with_e:
                    dec = e_de2(t1d)
                if with_a:
                    a_x('whx', xh, 0, 'wh_b')
                if with_e:
                    e_oh(dec, out_off)
                    t1p = e_p1()
                if with_a:
                    a_wc()
                if with_e:
                    e_p2(t1p, out_off)

            def copies():
                nc.vector.tensor_copy(comb[:, 0:SPB, 0:8], slow_c[...])
                nc.vector.tensor_copy(comb[:, 0:SPB, 8:24], m_c[...])
                nc.vector.tensor_copy(comb[:, 0:1, 24:40], comb[:, SPB:SPB + 1, 24:40])

            # ================= prologue: block 0 (+ A(1)) =================
            nc.vector.memset(h_car[...], 0.0)
            nc.vector.memset(comb[...], 0.0)
            nc.sync.dma_start(obs_sb[...], obsT[:, 0:COLS])
            a_sen()
            a_x('wzx', xzr, 0, 'wz_b')
            a_x('wrx', xzr, 16, 'wr_b')
            a_x('whx', xh, 0, 'wh_b')
            a_wc()                         # A(0)
            nc.sync.dma_start(obs_sb[...], obsT[:, COLS: 2 * COLS])
            fused_steps(with_int=False)    # B(0)
            stage_CEA(0, with_e=False)     # C(0) + copies + A(1)

            # ================= main loop: j in [1, NBLK) =================
            with tc.For_i(1, NBLK, 1,
                          hint_engines=(mybir.EngineType.PE, mybir.EngineType.DVE,
                                        mybir.EngineType.Activation)) as j:
                nc.sync.dma_start(obs_sb[...], obsT[:, ds((j + 1) * COLS, COLS)])
                HB = SPB // 2
                off = (j - 1) * COLS
                fillers = [lambda m=_m: c_group(m, 0, HB) for _m in range(5)]
                fillers += e1_fillers(off, 0, HB)
                fused_steps(fillers=fillers)         # B(j) + D(j-1) + C/E first halves
                for _m in range(5):
                    c_group(_m, HB, SPB)
                copies()
                stage_EA2(off)                       # E-h2(j-1) + A(j+1)

            # ================= epilogue: interaction + heads for last block =================
            _eoff = (NBLK - 1) * COLS
            fused_steps(with_gru=False, fillers=e1_fillers(_eoff, 0, SPB // 2))
            _HB = SPB // 2
            t1s = e_cmm('sck', 'sck_b', _HB, SPB)
            t1m = e_cmm('mck', 'mck_b', _HB, SPB)
            e_emm('sew', 'se_b', t1s, snch, _HB, SPB)
            e_emm('mew', 'me_b', t1m, mnch, _HB, SPB)
            t1d = e_dc2(_HB, SPB)
            dec = e_de2(t1d, _HB, SPB)
            e_oh(dec, _eoff, _HB, SPB)
            t1p = e_p1(_HB, SPB)
            e_p2(t1p, _eoff, _HB, SPB)
            nc.gpsimd.dma_start(finals[:, 0:8], snch[:, 0, SPB - 1])
            nc.gpsimd.dma_start(finals[:, 8:16], snch[:, 1, SPB - 1])
            nc.gpsimd.dma_start(finals[:, 16:24], mnch[:, 0, SPB - 1])
            nc.gpsimd.dma_start(finals[:, 24:32], mnch[:, 1, SPB - 1])
            nc.gpsimd.dma_start(finals[:, 32:40], comb[:, SPB, 24:32])
            nc.gpsimd.dma_start(finals[:, 40:48], comb[:, SPB, 32:40])

    nc.compile()
    return nc


# ======================= host-side packing =======================

def _tiles(Wm, kt, mt, mw=128):
    """[K, M] f32 -> [128, kt, mt, mw] bf16 lhsT tile grid (squeezed per caller)."""
    out = np.zeros((128, kt, mt, mw), np.float32)
    for k in range(kt):
        for m in range(mt):
            blk = Wm[k * 128:(k + 1) * 128, m * mw:(m + 1) * mw]
            out[:blk.shape[0], k, m, :blk.shape[1]] = blk
    return out.astype(BF16)


def _pack_weights(p):
    KEEP = np.r_[0:128, 256:768]
    phi = p['phi']
    w = {}
    w['we'] = _tiles(p['We'], 1, 2)[0:64, 0]                      # [64,2,128]
    w['wc'] = _tiles(p['Wc'], 2, 1)[:, :, 0]                      # [128,2,128]
    for nm, mat in (('wzx', p['Wz'][:D_M]), ('wrx', p['Wr'][:D_M]), ('whx', p['Wh'][:D_M]),
                    ('wzh', p['Wz'][D_M:]), ('wrh', p['Wr'][D_M:]), ('whh', p['Wh'][D_M:]),
                    ('wp', p['Wp'])):
        w[nm] = _tiles(mat, 2, 2)
    phism = np.concatenate([phi[0:128], phi[256:512]], axis=0)[:, KEEP]   # [384, 640]
    w['phism'] = _tiles(phism, 3, 5)
    w['phid'] = _tiles(phi[512:768][:, KEEP], 2, 5)
    w['dck'] = _tiles(p['Dc'][KEEP], 5, 1)[:, :, 0]
    w['sck'] = _tiles(p['Sc'][KEEP], 5, 1)[:, :, 0]
    w['mck'] = _tiles(p['Mc'][KEEP], 5, 1)[:, :, 0]
    w['sew'] = _tiles(p['Se'], 1, 2)[:, 0]
    w['mew'] = _tiles(p['Me'], 1, 2)[:, 0]
    w['dew'] = _tiles(p['De'], 1, 2)[:, 0]
    w['dc2'] = _tiles(p['dc'], 6, 1)[:, :, 0]
    w['de2'] = _tiles(p['de'], 1, 2)[:, 0]
    w['ohw'] = _tiles(p['oh'], 2, 1, mw=64)[:, :, 0]
    w['p1w'] = _tiles(p['p1'], 4, 1)[:, :, 0]
    w['p2w'] = _tiles(p['p2'], 1, 1, mw=64)[:, 0, 0]
    for k, sh in WSHAPES.items():
        assert list(w[k].shape) == sh, (k, w[k].shape, sh)
    return w


def _pack_biases(p):
    KEEP = np.r_[0:128, 256:768]
    bia = np.zeros((128, NBIAS), np.float32)

    def put(name, vec):
        vec = np.asarray(vec, np.float32)
        n = (len(vec) + 127) // 128
        for i in range(n):
            seg = vec[i * 128:(i + 1) * 128]
            bia[:len(seg), _BC[name] + i] = seg
    put('we_b', p['We_b']); put('wc_b', p['Wc_b'])
    put('wz_b', p['Wz_b']); put('wr_b', p['Wr_b']); put('wh_b', p['Wh_b'])
    put('phi_b', np.asarray(p['phi_b'])[KEEP])
    put('dck_b', p['Dc_b']); put('sck_b', p['Sc_b']); put('mck_b', p['Mc_b'])
    put('se_b', p['Se_b']); put('me_b', p['Me_b'])
    put('dc2_b', p['dc_b']); put('de2_b', p['de_b'])
    put('p1_b', p['p1_b']); put('oh_b', p['oh_b']); put('p2_b', p['p2_b'])
    return bia


def _pack_brows(p):
    # [k_row(=m tile), which(0=De_b,1=Wp_b), 128]: lhsT slice brows[:, w] is [2, 128]
    br = np.zeros((2, 2, 128), np.float32)
    for m in range(2):
        br[m, 0] = np.asarray(p['De_b'], np.float32)[m * 128:(m + 1) * 128]
        br[m, 1] = np.asarray(p['Wp_b'], np.float32)[m * 128:(m + 1) * 128]
    return br.astype(BF16)


_NC_CACHE = {}


def _get_nc(T_):
    if T_ not in _NC_CACHE:
        _NC_CACHE[T_] = build(T_)
    return _NC_CACHE[T_]


def kernel(obs_seq, params):
    obs = np.asarray(obs_seq, np.float32)
    p = {k: np.asarray(v, np.float32) for k, v in params.items()}
    Bf, Tf = obs.shape[0], obs.shape[1]
    nc = _get_nc(Tf)

    common = _pack_weights(p)
    common['biases'] = _pack_biases(p)
    common['ident'] = np.eye(128, dtype=np.float32).astype(BF16)
    selm = np.zeros((2, 16), np.float32)
    selm[0, 0:8] = 1.0
    selm[1, 8:16] = 1.0
    common['sel'] = selm.astype(BF16)
    common['brows'] = _pack_brows(p)

    in_maps = []
    for c in range(NCORES):
        shard = obs[c * BS:(c + 1) * BS]                      # [BS, T, D_S]
        obsT = shard.transpose(2, 1, 0).reshape(D_S, Tf * BS)  # col = t*BS + b
        pad = np.zeros((D_S, 2 * SPB * BS), np.float32)
        obsT = np.concatenate([obsT, pad], axis=1).astype(BF16)
        im = dict(common)
        im['obsT'] = obsT
        in_maps.append(im)

    res = run_bass_kernel_spmd(nc, in_maps, core_ids=list(range(NCORES)))

    actions = np.zeros((Bf, Tf, D_O), np.float32)
    preds = np.zeros((Bf, Tf, D_S), np.float32)
    S_last = np.zeros((Bf, D_ST), np.float32)
    M_last = np.zeros((Bf, D_ST), np.float32)
    D_last = np.zeros((Bf, D_ST), np.float32)
    for c in range(NCORES):
        r = res.results[c]
        sl = slice(c * BS, (c + 1) * BS)
        actions[sl] = r['actT'].reshape(D_O, Tf, BS).transpose(2, 1, 0)
        preds[sl] = r['predT'].reshape(D_S, Tf, BS).transpose(2, 1, 0)
        fin = r['finals']                                     # [128, 48]
        S_last[sl] = fin[:, 0:16].reshape(128, 2, BS).transpose(2, 1, 0).reshape(BS, 256)
        M_last[sl] = fin[:, 16:32].reshape(128, 2, BS).transpose(2, 1, 0).reshape(BS, 256)
        D_last[sl] = fin[:, 32:48].reshape(128, 2, BS).transpose(2, 1, 0).reshape(BS, 256)
    return actions, preds, S_last, M_last, D_last
